# revision 13
# baseline (speedup 1.0000x reference)
"""Trainium2 Bass kernel for nn_Density_Block (histogram_binning).

Computes, for N=1M rows:
    out1       = softmax(x @ weight + bias, axis=1)        [N, 101]
    out_interp = lerp of two adjacent bins of out1 at t*B   [N]

Strategy (8 NeuronCores, pure data parallel):
  * Host sorts rows by lower-bin index Li and interleaves sorted rows
    round-robin across cores, so every run of 5 consecutive 128-row tiles
    (on every core) gathers its two softmax bins from one 4-wide column
    window known at program-build time.
  * The matmul x@W+b runs as TWO fp16 matmuls accumulated in fp32 PSUM:
      MM1 (K=128): [ones; x_hi; x_lo(0:63)] @ [b_hi; W_hi; W_hi(0:63)]
      MM2 (K=65):  [ones; x_hi]             @ [b_lo; W_lo]
    (x_hi/W_hi fp16, *_lo = fp16 residuals; only feature 63's lo x W_hi
    cross-term is dropped -> ~1e-4 logit error.)
  * Bins padded 101 -> 102: pad logit = -100 (exp ~ 0), and the pad
    column carries out_interp so one DMA writes both outputs.
  * Per 10-tile group (2 PSUM banks): grouped exp on ScalarE, segmented
    softmax sums + interp dot on VectorE, normalize on the otherwise-idle
    GpSimd via a stride-0 broadcast multiply.
  * All DRAM I/O is partition-major ([128, ...] contiguous per
    partition) so every DMA is a 2D transfer with 2.5-4KB bursts; the
    host does the cheap reindexing.
"""

import os
import sys
import types
from contextlib import ExitStack

import numpy as np

import concourse.bass as bass
import concourse.tile as tile
from concourse import bacc, mybir
from concourse import bass_utils

F32 = mybir.dt.float32
F16 = mybir.dt.float16

N_CORES = 8
NBINS = 101
NB_PAD = 102
IND = 64
TILE = 128           # rows per tile (SBUF partitions)
TPB = 5              # tiles per PSUM bank (5 * 102 = 510 <= 512 floats)
BPG = 2              # PSUM banks per group
TPG = TPB * BPG      # tiles per group = 10
GROUP_ROWS = TILE * TPG          # 1280
WINDOW_TILES = TPG               # tiles sharing one gather window (whole group)
WIN = 4              # gather window width (needs intra-window Li spread <= 2)
PAD_LOGIT = -100.0
BANK = 512           # PSUM bank stride in f32 elements
K1 = 128             # MM1 contraction: ones + x_hi(64) + x_lo(0:63)
K2 = 65              # MM2 contraction: ones + x_hi(64)

LAST_RESULT = None   # stashed for the local test harness


def _install_ntff_hook():
    try:
        from antenv.axon_hooks import get_axon_ntff_profile_hook  # noqa: F401
        return
    except ImportError:
        pass
    try:
        import antenv
        from trn_agent_boot.trn_boot import _ntff_profile_via_ctypes
        mod = types.ModuleType("antenv.axon_hooks")
        hook = [_ntff_profile_via_ctypes("/opt/axon/libaxon_pjrt.so")]
        mod.set_axon_ntff_profile_hook = lambda h: hook.__setitem__(0, h)
        mod.get_axon_ntff_profile_hook = lambda: hook[0]
        sys.modules["antenv.axon_hooks"] = mod
        antenv.axon_hooks = mod
    except Exception:
        pass


def host_prepare(t, x, weight, bias, num_grid, n_cores=N_CORES):
    """Sort/shard/precompute on host.  Returns (meta, per-core input maps)."""
    t = np.ascontiguousarray(np.asarray(t, dtype=np.float32))
    x = np.asarray(x, dtype=np.float32)
    weight = np.asarray(weight, dtype=np.float32)
    bias = np.asarray(bias, dtype=np.float32)
    B = int(num_grid)
    N = t.shape[0]

    # Bin indices / interpolation weight, float32-exact vs the reference.
    tB = t * np.float32(B)
    U = np.ceil(tB)
    inter = np.float32(1.0) - (U - tB)
    L = U - np.float32(1.0)
    L = np.where(L < 0, L + np.float32(1.0), L)
    Li = L.astype(np.int32)
    Ui = U.astype(np.int32)

    chunk = n_cores * GROUP_ROWS
    NP = ((N + chunk - 1) // chunk) * chunk
    npad = NP - N
    R = NP // n_cores
    J = R // TILE                      # tiles per core
    n_windows = J // WINDOW_TILES
    winrows = TILE * n_cores * WINDOW_TILES

    perm = np.argsort(Li, kind="stable")
    Li_s = np.concatenate([Li[perm], np.full(npad, Li[perm[-1]] if N else 0, np.int32)])
    Ui_s = np.concatenate([Ui[perm], np.full(npad, 0, np.int32)])
    inter_s = np.concatenate([inter[perm], np.zeros(npad, np.float32)])

    LO = np.minimum(Li_s[::winrows], NB_PAD - WIN).astype(np.int32)
    assert LO.shape[0] == n_windows
    lo_per_row = np.repeat(LO, winrows)
    spread_ok = (Li_s - lo_per_row >= 0) & (np.maximum(Li_s, Ui_s) - lo_per_row < WIN)
    if not spread_ok[:N].all():
        bad = np.flatnonzero(~spread_ok[:N])[:5]
        raise AssertionError(f"gather-window assumption violated at sorted rows {bad}")

    coef_s = np.zeros((NP, WIN), np.float32)
    rows = np.arange(N)
    np.add.at(coef_s, (rows, (Li_s[:N] - lo_per_row[:N])), np.float32(1.0) - inter_s[:N])
    np.add.at(coef_s, (rows, (Ui_s[:N] - lo_per_row[:N])), inter_s[:N])

    # Weight/bias fp16 hi/lo, padded bins.
    wb = np.zeros((IND + 1, NB_PAD), np.float32)   # row 0..63 = W, row 64 = bias
    wb[:IND, :NBINS] = weight
    wb[IND, :NBINS] = bias
    wb[IND, NBINS] = np.float32(PAD_LOGIT)
    w_hi = wb.astype(np.float16)
    w_lo = (wb - w_hi.astype(np.float32)).astype(np.float16)
    # MM1 rhs [128, 102]: [b_hi; W_hi; W_hi(0:63)]
    w1 = np.empty((K1, NB_PAD), np.float16)
    w1[0] = w_hi[IND]
    w1[1:IND + 1] = w_hi[:IND]
    w1[IND + 1:] = w_hi[:IND - 1]
    # MM2 rhs [65, 102]: [b_lo; W_lo]
    w2 = np.empty((K2, NB_PAD), np.float16)
    w2[0] = w_lo[IND]
    w2[1:] = w_lo[:IND]

    # Sorted+padded x -> per-core fp16 stack [128, R]:
    # partition 0 = ones, 1..64 = x_hi, 65..127 = x_lo(features 0:63).
    xs = np.zeros((NP, IND), np.float32)
    xs[:N] = x[perm]
    in_maps = []
    for i in range(n_cores):
        xi = xs[i::n_cores]                       # [R, 64] f32
        xi_hi = xi.astype(np.float16)
        xi_lo = (xi - xi_hi.astype(np.float32)).astype(np.float16)
        xst = np.empty((TILE, R), np.float16)
        xst[0] = np.float16(1.0)
        xst[1:IND + 1] = xi_hi.T
        xst[IND + 1:] = xi_lo[:, :IND - 1].T
        # coef partition-major: [128, J*4]
        ci = coef_s[i::n_cores].reshape(J, TILE, WIN).transpose(1, 0, 2) \
            .reshape(TILE, J * WIN)
        in_maps.append({
            "xst": xst,
            "w1": w1,
            "w2": w2,
            "coef": np.ascontiguousarray(ci),
        })

    meta = dict(N=N, NP=NP, R=R, J=J, LO=LO, perm=perm, n_cores=n_cores)
    return meta, in_maps


def build_program(LO, R, n_cores=N_CORES):
    """Build + compile the (SPMD-identical) Bass program for one core."""
    J = R // TILE
    n_groups = R // GROUP_ROWS
    assert n_groups * GROUP_ROWS == R
    assert len(LO) == J // WINDOW_TILES

    nc = bacc.Bacc("TRN2", target_bir_lowering=False, debug=False,
                   num_devices=n_cores)
    xst = nc.dram_tensor("xst", [TILE, R], F16, kind="ExternalInput").ap()
    w1 = nc.dram_tensor("w1", [K1, NB_PAD], F16, kind="ExternalInput").ap()
    w2 = nc.dram_tensor("w2", [K2, NB_PAD], F16, kind="ExternalInput").ap()
    coef = nc.dram_tensor("coef", [TILE, J * WIN], F32, kind="ExternalInput").ap()
    # Partition-major combined output: [128, J*102]; per tile j cols
    # j*102 .. j*102+101 = out1 row block, col j*102+101 = out_interp.
    comb = nc.dram_tensor("comb", [TILE, J * NB_PAD], F32, kind="ExternalOutput").ap()

    Exp = mybir.ActivationFunctionType.Exp
    mult = mybir.AluOpType.mult
    add = mybir.AluOpType.add
    X = mybir.AxisListType.X

    with tile.TileContext(nc) as tc:
        with ExitStack() as ctx:
            wpool = ctx.enter_context(tc.tile_pool(name="w", bufs=1))
            xpool = ctx.enter_context(tc.tile_pool(name="x", bufs=6))
            cpool = ctx.enter_context(tc.tile_pool(name="c", bufs=6))
            ppool = ctx.enter_context(tc.tile_pool(name="ps", bufs=4, space="PSUM"))
            epool = ctx.enter_context(tc.tile_pool(name="ex", bufs=6))
            opool = ctx.enter_context(tc.tile_pool(name="o1", bufs=6))
            spool = ctx.enter_context(tc.tile_pool(name="sm", bufs=8))
            tpool = ctx.enter_context(tc.tile_pool(name="tt", bufs=8))

            w1t = wpool.tile([K1, NB_PAD], F16)
            nc.sync.dma_start(w1t[:], w1[:])
            w2t = wpool.tile([K2, NB_PAD], F16)
            nc.sync.dma_start(w2t[:], w2[:])

            for g in range(n_groups):
                c0 = g * GROUP_ROWS          # column offset into xst
                xt = xpool.tile([TILE, GROUP_ROWS], F16)
                nc.scalar.dma_start(xt[:], xst[:, c0:c0 + GROUP_ROWS])
                cf = cpool.tile([TILE, TPG * WIN], F32)
                nc.sync.dma_start(cf[:], coef[:, g * TPG * WIN:(g + 1) * TPG * WIN])

                ps = ppool.tile([128, BPG * BANK], F32)
                for ti in range(TPG):
                    o = (ti // TPB) * BANK + (ti % TPB) * NB_PAD
                    xsl = slice(ti * TILE, (ti + 1) * TILE)
                    nc.tensor.matmul(ps[:, o:o + NB_PAD], lhsT=xt[:, xsl],
                                     rhs=w1t[:], start=True, stop=False)
                    nc.tensor.matmul(ps[:, o:o + NB_PAD], lhsT=xt[0:K2, xsl],
                                     rhs=w2t[:], start=False, stop=True)

                ex = epool.tile([128, TPG * NB_PAD], F32)
                nc.scalar.activation(
                    ex[:].rearrange("p (b c) -> p b c", b=BPG),
                    ps[:].rearrange("p (b c) -> p b c", b=BPG)[:, :, 0:TPB * NB_PAD],
                    Exp,
                )

                sg = spool.tile([128, TPG], F32)
                nc.vector.tensor_reduce(
                    sg[:],
                    ex[:].rearrange("p (t c) -> p t c", t=TPG),
                    axis=X, op=add,
                )
                rg = spool.tile([128, TPG], F32)
                nc.vector.reciprocal(rg[:], sg[:])

                # interp path (independent of the normalize): works on the
                # unnormalized exp window, scaled by 1/s at the end.
                lo = int(LO[g])
                tt = tpool.tile([128, TPG * WIN], F32)
                nc.vector.tensor_tensor(
                    tt[:].rearrange("p (t c) -> p t c", t=TPG),
                    ex[:].rearrange("p (t c) -> p t c", t=TPG)[:, :, lo:lo + WIN],
                    cf[:].rearrange("p (t c) -> p t c", t=TPG),
                    op=mult,
                )
                ri = spool.tile([128, TPG], F32)
                nc.vector.tensor_reduce(
                    ri[:], tt[:].rearrange("p (t c) -> p t c", t=TPG),
                    axis=X, op=add,
                )
                o1 = opool.tile([128, TPG * NB_PAD], F32)
                nc.vector.tensor_tensor(
                    o1[:].rearrange("p (t c) -> p t c", t=TPG)[:, :, NBINS:NB_PAD],
                    ri[:].broadcast_to((128, TPG, 1)),
                    rg[:].broadcast_to((128, TPG, 1)),
                    op=mult,
                )

                # normalize the real 101 bins on GpSimd
                nc.gpsimd.tensor_tensor(
                    o1[:].rearrange("p (t c) -> p t c", t=TPG)[:, :, 0:NBINS],
                    ex[:].rearrange("p (t c) -> p t c", t=TPG)[:, :, 0:NBINS],
                    rg[:].broadcast_to((128, TPG, NBINS)),
                    op=mult,
                )

                nc.sync.dma_start(
                    comb[:, g * TPG * NB_PAD:(g + 1) * TPG * NB_PAD], o1[:])

    nc.compile()
    return nc


def kernel(t, x, weight, bias, num_grid):
    global LAST_RESULT
    trace = bool(os.environ.get("BASS_TRACE"))
    if trace:
        _install_ntff_hook()
        bass_utils.upload_artifacts = lambda tmpdir: "local://" + tmpdir

    meta, in_maps = host_prepare(t, x, weight, bias, num_grid)
    nc = build_program(meta["LO"], meta["R"], meta["n_cores"])

    res = bass_utils.run_bass_kernel_spmd(
        nc, in_maps, core_ids=list(range(meta["n_cores"])), trace=trace,
    )
    LAST_RESULT = res

    N, NP, n_cores = meta["N"], meta["NP"], meta["n_cores"]
    R, J = meta["R"], meta["J"]
    perm = meta["perm"]
    comb_s = np.empty((NP, NB_PAD), np.float32)
    for i in range(n_cores):
        ci = res.results[i]["comb"].reshape(TILE, J, NB_PAD)
        comb_s[i::n_cores] = ci.transpose(1, 0, 2).reshape(R, NB_PAD)
    out1 = np.empty((N, NBINS), np.float32)
    oint = np.empty((N,), np.float32)
    out1[perm] = comb_s[:N, :NBINS]
    oint[perm] = comb_s[:N, NBINS]
    return out1, oint
